# revision 14
# baseline (speedup 1.0000x reference)
"""Trainium2 Bass kernel for deformable attention.

Contract: kernel(**inputs) takes the FULL inputs (as produced by the problem's
setup_inputs) and returns the FULL [4, 1024, 256] float32 output. Internally the
work is sharded over 8 NeuronCores: core c handles batch c//2 and query half
c%2 (512 queries), with the batch's full value feature map replicated on the
core.

Per-core pipeline (all shapes hardcoded for B=4, Q=1024, D=256, H=W=128,
nh=8, npts=4):
  1. The value projection W_v commutes past the (linear) bilinear/attention
     reduce, so it is folded into the output projection on the host:
     Wcomb_h = W_v @ W_out_h and bvW_h = b_v @ W_out_h, with a per-(q,h)
     sum-of-weights term correcting the bias at zero-padded borders. The
     kernel therefore gathers raw value rows -- no feature-map GEMM.
  2. The value map ships as fp8 e3m4 scaled by 2 (quantization rel err ~1.3%
     end to end, under the 2e-2 gate); the 1/2 is folded into Wcomb. This
     halves gather DMA traffic vs bf16.
  3. Coefficient chain: offsets/attention GEMMs + softmax + bilinear weight
     computation, in [query-partition, sample-free] layout, fp32. Per q-tile
     of 128 queries; gather indices for a tile ship as soon as it finishes.
     All cross-phase intermediates are per-q-tile tiles so chunk 0's gather
     only waits on q-tile 0's index path.
  4. Gather: per (query, head, point, row-corner) descriptor, one dma_gather
     element of 512 fp8 values = two adjacent columns at one row of the
     value map. 2048 idxs per call, 16 calls.
  5. Weighted reduce on the TensorEngine: the 128 gathered slots of a query
     pair are the contraction dim (gathered tile is the fp8 stationary
     operand); the moving operand is a masked block-diagonal [128, 16] bf16
     weight matrix built from bilinear*attention weights. Output lands as
     [d, (q, h)] in PSUM, which is exactly the lhsT layout the final GEMM
     needs. PSUM->SBUF staging runs on the Activation engine to keep DVE off
     the critical path.
  6. out = weighted @ Wcomb + sw * bvW + b_out, stored as bf16; each q-tile's
     output GEMM issues right after its last gather chunk so the PE overlaps
     it with later chunks' DMA.
"""

from contextlib import ExitStack

import numpy as np
import ml_dtypes

NH, NPTS = 8, 4
D = 256
HW = 128            # H == W == 128
NROWS = HW * HW     # 16384
QPC = 512           # queries per core
NCORES = 8
NPAIRS = QPC // 2   # 256 query pairs
NCHUNK = 16         # gather chunks
PAIRS_PER_CHUNK = NPAIRS // NCHUNK       # 16
IDX_PER_CHUNK = PAIRS_PER_CHUNK * 128    # 2048
CHUNKS_PER_QT = NCHUNK // 4              # 4
VSCALE = 2.0        # value map pre-scale folded into Wcomb

_CACHE = {}


def _mask16_np():
    """[128, 16] bf16: mask[qq*64 + h*8 + p*2 + yp, qq*8 + h] = 1."""
    m = np.zeros((128, 16), dtype=np.float32)
    for qq in range(2):
        for h in range(NH):
            for p in range(NPTS):
                for yp in range(2):
                    m[qq * 64 + h * 8 + p * 2 + yp, qq * 8 + h] = 1.0
    return m.astype(ml_dtypes.bfloat16)


def _rep_np():
    """[64, 4, 128] f32: rep[k, g4, 16r+t] = 1 iff k == 16*g4 + t."""
    m = np.zeros((64, 4, 128), dtype=np.float32)
    for g4 in range(4):
        for r in range(8):
            for t in range(16):
                m[16 * g4 + t, g4, 16 * r + t] = 1.0
    return m


def _build_bass():
    import concourse.bass as bass
    import concourse.bacc as bacc
    import concourse.mybir as mybir
    import concourse.tile as tile
    from concourse.masks import make_identity

    f32 = mybir.dt.float32
    bf16 = mybir.dt.bfloat16
    f8 = mybir.dt.float8e3
    i16 = mybir.dt.int16
    i32 = mybir.dt.int32
    Alu = mybir.AluOpType
    Act = mybir.ActivationFunctionType

    nc = bacc.Bacc("TRN2", target_bir_lowering=False,
                   dynamic_dma_scratch_size=65536)

    # ---- I/O ----
    query = nc.dram_tensor("query", [QPC, D], f32, kind="ExternalInput")
    refp = nc.dram_tensor("reference_points", [QPC, 2], f32, kind="ExternalInput")
    value = nc.dram_tensor("value", [NROWS, D], f8, kind="ExternalInput")
    W_off = nc.dram_tensor("W_off", [D, 64], f32, kind="ExternalInput")
    b_off = nc.dram_tensor("b_off", [64], f32, kind="ExternalInput")
    W_attn = nc.dram_tensor("W_attn", [D, 32], f32, kind="ExternalInput")
    b_attn = nc.dram_tensor("b_attn", [32], f32, kind="ExternalInput")
    Wcomb = nc.dram_tensor("Wcomb", [NH * D, D], bf16, kind="ExternalInput")
    bvW = nc.dram_tensor("bvW", [NH, D], bf16, kind="ExternalInput")
    b_out = nc.dram_tensor("b_out", [D], f32, kind="ExternalInput")
    out = nc.dram_tensor("out", [QPC, D], bf16, kind="ExternalOutput")

    mask_dram = nc.inline_tensor(_mask16_np(), name="mask16")
    rep_dram = nc.inline_tensor(_rep_np(), name="rep64")

    with tile.TileContext(nc) as tc, ExitStack() as ctx:
        sb = ctx.enter_context(tc.tile_pool(name="sb", bufs=1))
        ps = ctx.enter_context(tc.tile_pool(name="ps", bufs=1, space="PSUM"))

        # ---- queries for tile 0 first: they head the critical path ----
        q_sbs, rps = [], []
        for qt in range(4):
            q_sbs.append(sb.tile([128, 256], f32, tag=f"q_sb{qt}", name=f"q_sb{qt}"))
            rps.append(sb.tile([128, 2], f32, tag=f"rp{qt}", name=f"rp{qt}"))
        nc.sync.dma_start(q_sbs[0][:], query[0:128, :])
        nc.sync.dma_start(rps[0][:], refp[0:128, :])

        wcat = sb.tile([128, 2, 96], f32, tag="wcat")
        nc.sync.dma_start(wcat[:, :, 0:64], W_off[:].rearrange("(t p) n -> p t n", p=128))
        nc.sync.dma_start(wcat[:, :, 64:96], W_attn[:].rearrange("(t p) n -> p t n", p=128))
        nc.vector.tensor_scalar_mul(wcat[:, :, 0:64], wcat[:, :, 0:64], 0.1)
        bias_cat = sb.tile([1, 96], f32, tag="bias_cat")
        nc.sync.dma_start(bias_cat[:, 0:64], b_off[None, :])
        nc.sync.dma_start(bias_cat[:, 64:96], b_attn[None, :])
        nc.vector.tensor_scalar_mul(bias_cat[:, 0:64], bias_cat[:, 0:64], 0.1)

        # wout's 2.9us transfer must land inside the lead-in DMA idle, so it
        # goes on the SP queue right behind the critical-path loads.
        wout_bf = sb.tile([128, 16, 256], bf16, tag="wout")
        nc.sync.dma_start(wout_bf[:], Wcomb[:].rearrange("(t p) n -> p t n", p=128))

        for qt in range(1, 4):
            qsl = slice(qt * 128, (qt + 1) * 128)
            nc.sync.dma_start(q_sbs[qt][:], query[qsl, :])
            nc.sync.dma_start(rps[qt][:], refp[qsl, :])

        ident = sb.tile([128, 128], f32, tag="ident")
        make_identity(nc, ident[:])
        ones1 = sb.tile([1, 128], f32, tag="ones1")
        nc.vector.memset(ones1[:], 1.0)
        # warm the activation table off the critical path
        actwarm = sb.tile([1, 128], f32, tag="actwarm")
        nc.scalar.activation(actwarm[:], ones1[:], Act.Exp)
        mask16 = sb.tile([128, 16], bf16, tag="mask16")
        nc.sync.dma_start(mask16[:], mask_dram[:])
        rep64 = sb.tile([64, 4, 128], f32, tag="rep64")
        nc.sync.dma_start(rep64[:], rep_dram[:])
        bvw_bf = sb.tile([8, 256], bf16, tag="bvw")
        nc.sync.dma_start(bvw_bf[:], bvW[:])
        bout_sb = sb.tile([1, 256], f32, tag="bout")
        nc.sync.dma_start(bout_sb[:], b_out[None, :])

        # per-q-tile persistent intermediates (split so consumers only wait
        # on their own tile's producers)
        idxts = [sb.tile([128, 64, 2, 4], i16, tag=f"idxt{qt}", name=f"idxt{qt}")
                 for qt in range(4)]
        w_a_is = [sb.tile([128, 64], bf16, tag=f"w_a{qt}", name=f"w_a{qt}")
                  for qt in range(4)]
        w_b_is = [sb.tile([128, 64], bf16, tag=f"w_b{qt}", name=f"w_b{qt}")
                  for qt in range(4)]
        reds = [sb.tile([128, 2, 128, 8], bf16, tag=f"red{qt}", name=f"red{qt}")
                for qt in range(4)]
        swTs = [sb.tile([8, 128], bf16, tag=f"swT{qt}", name=f"swT{qt}")
                for qt in range(4)]

        # ================= coefficient phase (4 q-tiles of 128) =============
        # per q-tile: offsets GEMM + gather-index path (ships indices as
        # early as possible), weight path as a separate pass; interleaved so
        # late tiles' indices land before the gather stream catches up while
        # tile 0's weights are still ready before its first reduce.
        qt_state = {}

        def idx_pass(qt):
            q_sb, rp = q_sbs[qt], rps[qt]

            pst = ps.tile([128, 256], f32, tag="tp", bufs=2)
            nc.tensor.transpose(pst[:, 0:128], q_sb[:, 0:128], ident[:])
            nc.tensor.transpose(pst[:, 128:256], q_sb[:, 128:256], ident[:])
            qT = sb.tile([128, 2, 128], f32, tag="qT", bufs=2)
            nc.scalar.copy(qT[:], pst[:])

            psc = ps.tile([128, 96], f32, tag="tp", bufs=2)
            nc.tensor.matmul(psc[:], qT[:, 0, :], wcat[:, 0, :], start=True, stop=False)
            nc.tensor.matmul(psc[:], qT[:, 1, :], wcat[:, 1, :], start=False, stop=False)
            nc.tensor.matmul(psc[:], ones1[:], bias_cat[:], start=False, stop=True)
            coef = sb.tile([128, 96], f32, tag="coef_sb", bufs=4)
            nc.scalar.copy(coef[:], psc[:])

            # sampling grid -> bilinear weights, x/y interleaved [128, 32, 2]
            t_u = sb.tile([128, 32, 2], f32, tag="t_u", bufs=4)
            nc.vector.tensor_tensor(
                t_u[:], coef[:, 0:64].rearrange("p (s c) -> p s c", c=2),
                rp[:, None, :].to_broadcast([128, 32, 2]), Alu.add)
            t_c = t_u  # in-place ok per-element
            nc.vector.tensor_scalar(t_c[:], t_u[:], 0.0, 1.0, Alu.max, Alu.min)
            pxs = sb.tile([128, 64], f32, tag="pxs", bufs=4)  # px + 128
            nc.vector.tensor_scalar(pxs[:], t_c[:].rearrange("p a b -> p (a b)"),
                                    128.0, 127.5, Alu.mult, Alu.add)
            # floor(pxs) = round(pxs - 0.5): pxs is positive and lands on
            # k+0.5 only at the clip boundaries 127.5/255.5, where round
            # gives 127/255 = floor as required.
            ph = sb.tile([128, 64], f32, tag="ph", bufs=4)
            nc.vector.tensor_scalar_add(ph[:], pxs[:], -0.5)
            ri = sb.tile([128, 64], i32, tag="ri", bufs=4)
            nc.vector.tensor_copy(ri[:], ph[:])
            flr = sb.tile([128, 64], f32, tag="flr", bufs=4)  # floor(px) + 128
            nc.vector.tensor_copy(flr[:], ri[:])
            st = sb.tile([128, 64], f32, tag="st", bufs=4)    # clip start + 128
            nc.vector.tensor_scalar(st[:], flr[:], 128.0, 254.0, Alu.max, Alu.min)
            # ---- gather-index path first: this q-tile's gathers can start
            # while the weight path below is still computing ----
            tbase = sb.tile([128, 32], f32, tag="tbase", bufs=4)
            nc.vector.tensor_scalar(
                tbase[:], st[:].rearrange("p (s c) -> p s c", c=2)[:, :, 1],
                128.0, -16512.0, Alu.mult, Alu.add)
            idx64 = sb.tile([128, 32, 2], f32, tag="idx64", bufs=4)
            nc.vector.tensor_tensor(idx64[:, :, 0], tbase[:],
                                    st[:].rearrange("p (s c) -> p s c", c=2)[:, :, 0], Alu.add)
            nc.vector.tensor_scalar_add(idx64[:, :, 1], idx64[:, :, 0], 128.0)
            # transpose idx to [slot, q], then replicate to all 8
            # 16-partition blocks (one per Q7 core) with 0/1 REP matmuls:
            # out[16r+t, q] = ivT[16*g4+t, q]
            iv = idx64[:].rearrange("p s c -> p (s c)")
            psv = ps.tile([64, 128], f32, tag="psv", bufs=1)
            nc.tensor.transpose(psv[:], iv[:], ident[:])
            ivT = sb.tile([64, 128], f32, tag="ivT", bufs=2)
            nc.vector.tensor_copy(ivT[:], psv[:])
            psi = ps.tile([128, 4, 128], f32, tag="psi", bufs=1)
            for g4 in range(4):
                nc.tensor.matmul(psi[:, g4, :], rep64[:, g4, :], ivT[:],
                                 start=True, stop=True)
            # one DVE copy: [128, (g4, j, qq)] f32 -> idxt[., j, qq, g4] i16
            nc.vector.tensor_copy(
                idxts[qt][:],
                psi[:].rearrange("p g (j q) -> p j q g", q=2))
            qt_state[qt] = (coef, pxs, flr, st)

        def weight_pass(qt):
            coef, pxs, flr, st = qt_state[qt]
            # softmax + bilinear weights
            expw = sb.tile([128, 8, 4], f32, tag="expw", bufs=4)
            nc.scalar.activation(expw[:], coef[:, 64:96], Act.Exp)
            den = sb.tile([128, 8], f32, tag="den", bufs=4)
            nc.vector.tensor_reduce(den[:], expw[:], axis=mybir.AxisListType.X, op=Alu.add)
            rden = sb.tile([128, 8], f32, tag="rden", bufs=4)
            nc.vector.reciprocal(rden[:], den[:])
            attn = sb.tile([128, 32], f32, tag="attn", bufs=4)
            nc.vector.tensor_tensor(
                attn[:].rearrange("p (h f) -> p h f", f=4), expw[:],
                rden[:, :, None].to_broadcast([128, 8, 4]), Alu.mult)
            w1 = sb.tile([128, 64], f32, tag="w1", bufs=4)
            nc.vector.tensor_tensor(w1[:], pxs[:], flr[:], Alu.subtract)
            dd = sb.tile([128, 64], f32, tag="dd", bufs=4)
            nc.vector.tensor_tensor(dd[:], flr[:], st[:], Alu.subtract)
            m0 = sb.tile([128, 64], f32, tag="m0", bufs=4)
            nc.vector.tensor_scalar(m0[:], dd[:], 0.0, None, Alu.is_equal)
            mneg = sb.tile([128, 64], f32, tag="mneg", bufs=4)
            nc.vector.tensor_scalar(mneg[:], dd[:], -1.0, None, Alu.is_equal)
            mpos = sb.tile([128, 64], f32, tag="mpos", bufs=4)
            nc.vector.tensor_scalar(mpos[:], dd[:], 1.0, None, Alu.is_equal)
            u0 = sb.tile([128, 64], f32, tag="u0", bufs=4)
            nc.vector.tensor_scalar(u0[:], w1[:], 1.0, -1.0, Alu.subtract, Alu.mult)
            # wA = u0*m0 + u1*mneg ; wB = u1*m0 + u0*mpos   (u1 == w1)
            tA = sb.tile([128, 64], f32, tag="tA", bufs=4)
            nc.vector.tensor_tensor(tA[:], u0[:], m0[:], Alu.mult)
            tB = sb.tile([128, 64], f32, tag="tB", bufs=4)
            nc.vector.tensor_tensor(tB[:], w1[:], mneg[:], Alu.mult)
            wA = sb.tile([128, 32, 2], f32, tag="wA", bufs=4)
            nc.vector.tensor_tensor(wA[:].rearrange("p a b -> p (a b)"), tA[:], tB[:], Alu.add)
            nc.vector.tensor_tensor(tA[:], w1[:], m0[:], Alu.mult)
            nc.vector.tensor_tensor(tB[:], u0[:], mpos[:], Alu.mult)
            wB = sb.tile([128, 32, 2], f32, tag="wB", bufs=4)
            nc.vector.tensor_tensor(wB[:].rearrange("p a b -> p (a b)"), tA[:], tB[:], Alu.add)

            # combine with attention; build wab [128, (AB, h*p, yp)]
            aw = sb.tile([128, 32], f32, tag="aw", bufs=4)
            nc.vector.tensor_tensor(aw[:], attn[:], wA[:, :, 0], Alu.mult)
            bw = sb.tile([128, 32], f32, tag="bw", bufs=4)
            nc.vector.tensor_tensor(bw[:], attn[:], wB[:, :, 0], Alu.mult)
            vcat = sb.tile([128, 32, 2], f32, tag="vcat", bufs=4)
            nc.vector.tensor_copy(vcat[:, :, 0], wA[:, :, 1])
            nc.vector.tensor_copy(vcat[:, :, 1], wB[:, :, 1])
            wab = sb.tile([128, 2, 32, 2], f32, tag="wab", bufs=4)
            nc.vector.tensor_tensor(wab[:, 0], vcat[:],
                                    aw[:, :, None].to_broadcast([128, 32, 2]), Alu.mult)
            nc.vector.tensor_tensor(wab[:, 1], vcat[:],
                                    bw[:, :, None].to_broadcast([128, 32, 2]), Alu.mult)

            # sum of all weights per (q, h) -- border-clip correction for the
            # folded b_v term: sw = sum_{AB,p,yp} wab
            swq = sb.tile([128, 8], f32, tag="swq", bufs=4)
            nc.vector.tensor_reduce(
                swq[:], wab[:].rearrange("p a (h r) c -> p h a r c", h=8),
                axis=mybir.AxisListType.XYZ, op=Alu.add)

            pst2b = ps.tile([128, 256], f32, tag="tp", bufs=2)
            nc.tensor.transpose(pst2b[:, 0:128],
                                wab[:].rearrange("p a s c -> p (a s c)"), ident[:])
            pst3 = ps.tile([8, 128], f32, tag="img", bufs=2)
            nc.tensor.transpose(pst3[:], swq[:], ident[:])
            wabT = sb.tile([128, 128], f32, tag="wabT", bufs=2)
            nc.scalar.copy(wabT[:], pst2b[:, 0:128])
            nc.scalar.copy(swTs[qt][:], pst3[:])
            nc.vector.tensor_copy(w_a_is[qt][0:64, :], wabT[0:64, 0:128:2])
            nc.vector.tensor_copy(w_a_is[qt][64:128, :], wabT[0:64, 1:128:2])
            nc.vector.tensor_copy(w_b_is[qt][0:64, :], wabT[64:128, 0:128:2])
            nc.vector.tensor_copy(w_b_is[qt][64:128, :], wabT[64:128, 1:128:2])

        idx_pass(0)
        idx_pass(1)
        weight_pass(0)
        idx_pass(2)
        weight_pass(1)
        idx_pass(3)
        weight_pass(2)
        weight_pass(3)

        # ================= gather + weighted reduce =========================
        import concourse.bass as bass_mod
        gather_src = bass_mod.AP(
            tensor=value, offset=0, ap=[[256, NROWS - 1], [1, 512]])

        def final_gemm(qt):
            pso = ps.tile([128, 256], f32, tag="img", bufs=2)
            for kt in range(16):
                h, dh = kt // 2, kt % 2
                lhsT = reds[qt][:, dh, :, h]
                nc.tensor.matmul(pso[:], lhsT, wout_bf[:, kt, :],
                                 start=(kt == 0), stop=False)
            nc.tensor.matmul(pso[:], swTs[qt][:], bvw_bf[:],
                             start=False, stop=False)
            nc.tensor.matmul(pso[:], ones1[:], bout_sb[:], start=False, stop=True)
            o_sb = sb.tile([128, 256], bf16, tag="o_sb", bufs=2)
            nc.scalar.copy(o_sb[:], pso[:])
            nc.sync.dma_start(out[qt * 128:(qt + 1) * 128, :], o_sb[:])

        for g in range(NCHUNK):
            qt, gq = g // CHUNKS_PER_QT, g % CHUNKS_PER_QT
            gsl = slice(gq * PAIRS_PER_CHUNK, (gq + 1) * PAIRS_PER_CHUNK)
            gt_sb = sb.tile([128, PAIRS_PER_CHUNK, 512], f8, tag="gat", bufs=5)
            nc.gpsimd.dma_gather(
                out_ap=gt_sb[:],
                in_ap=gather_src,
                idxs_ap=idxts[qt][:].rearrange("p a b c -> p (a b c)")[
                    :, gq * (IDX_PER_CHUNK // 16):(gq + 1) * (IDX_PER_CHUNK // 16)],
                num_idxs=IDX_PER_CHUNK,
                num_idxs_reg=IDX_PER_CHUNK,
                elem_size=512,
                elem_step=256,
            )
            wblkA = sb.tile([128, PAIRS_PER_CHUNK, 16], bf16, tag="wblkA", bufs=4)
            nc.vector.tensor_tensor(
                wblkA[:], mask16[:, None, :].to_broadcast([128, PAIRS_PER_CHUNK, 16]),
                w_a_is[qt][:, gsl, None].to_broadcast([128, PAIRS_PER_CHUNK, 16]),
                Alu.mult)
            wblkB = sb.tile([128, PAIRS_PER_CHUNK, 16], bf16, tag="wblkB", bufs=4)
            nc.vector.tensor_tensor(
                wblkB[:], mask16[:, None, :].to_broadcast([128, PAIRS_PER_CHUNK, 16]),
                w_b_is[qt][:, gsl, None].to_broadcast([128, PAIRS_PER_CHUNK, 16]),
                Alu.mult)

            pred = ps.tile([128, 2, PAIRS_PER_CHUNK * 16], f32, tag="red_ps", bufs=2)
            for j in range(PAIRS_PER_CHUNK):
                osl = slice(j * 16, (j + 1) * 16)
                nc.tensor.matmul(pred[:, 0, osl], gt_sb[:, j, 0:128], wblkA[:, j, :],
                                 start=True, stop=False)
                nc.tensor.matmul(pred[:, 1, osl], gt_sb[:, j, 128:256], wblkA[:, j, :],
                                 start=True, stop=False)
                nc.tensor.matmul(pred[:, 0, osl], gt_sb[:, j, 256:384], wblkB[:, j, :],
                                 start=False, stop=True)
                nc.tensor.matmul(pred[:, 1, osl], gt_sb[:, j, 384:512], wblkB[:, j, :],
                                 start=False, stop=True)
            qsl2 = slice(gq * 2 * PAIRS_PER_CHUNK, (gq + 1) * 2 * PAIRS_PER_CHUNK)
            nc.scalar.copy(
                reds[qt][:, :, qsl2, :],
                pred[:].rearrange("p d (a b) -> p d a b", b=8))
            if gq == CHUNKS_PER_QT - 1:
                final_gemm(qt)

    nc.compile()
    return nc


def _get_nc():
    if "nc" not in _CACHE:
        _CACHE["nc"] = _build_bass()
    return _CACHE["nc"]


def _make_in_maps(inputs):
    query = np.ascontiguousarray(np.asarray(inputs["query"], dtype=np.float32))
    refp = np.ascontiguousarray(np.asarray(inputs["reference_points"], dtype=np.float32))
    value = np.ascontiguousarray(
        (np.asarray(inputs["value"], dtype=np.float32) * VSCALE
         ).astype(ml_dtypes.float8_e3m4))
    consts = {
        k: np.ascontiguousarray(np.asarray(inputs[k], np.float32))
        for k in ["W_off", "b_off", "W_attn", "b_attn", "b_out"]
    }
    W_v = np.asarray(inputs["W_v"], np.float64)
    b_v = np.asarray(inputs["b_v"], np.float64)
    W_out = np.asarray(inputs["W_out"], np.float64).reshape(NH, D, D)
    consts["Wcomb"] = np.ascontiguousarray(
        (np.einsum("ij,hjk->hik", W_v, W_out) / VSCALE
         ).reshape(NH * D, D).astype(ml_dtypes.bfloat16))
    consts["bvW"] = np.ascontiguousarray(
        np.einsum("j,hjk->hk", b_v, W_out).astype(ml_dtypes.bfloat16))
    in_maps = []
    for c in range(NCORES):
        b, s = c // 2, c % 2
        qsl = slice(s * QPC, (s + 1) * QPC)
        in_maps.append({
            "query": np.ascontiguousarray(query[b, qsl]),
            "reference_points": np.ascontiguousarray(refp[b, qsl]),
            "value": np.ascontiguousarray(value[b]),
            **consts,
        })
    return in_maps


def _assemble(outs, shape):
    out = np.zeros(shape, dtype=np.float32)
    for c in range(NCORES):
        b, s = c // 2, c % 2
        out[b, s * QPC:(s + 1) * QPC] = np.asarray(outs[c]["out"], dtype=np.float32)
    return out


def kernel(query, reference_points, value, W_off, b_off, W_attn, b_attn,
           W_v, b_v, W_out, b_out, H=128, W=128, **_unused):
    assert int(H) == HW and int(W) == HW
    from concourse.bass_utils import run_bass_kernel_spmd

    inputs = dict(query=query, reference_points=reference_points, value=value,
                  W_off=W_off, b_off=b_off, W_attn=W_attn, b_attn=b_attn,
                  W_v=W_v, b_v=b_v, W_out=W_out, b_out=b_out)
    in_maps = _make_in_maps(inputs)
    nc = _get_nc()
    res = run_bass_kernel_spmd(nc, in_maps, core_ids=list(range(NCORES)))
    outs = res.results if hasattr(res, "results") else res
    B, Q, _ = np.asarray(query).shape
    return _assemble(outs, (B, Q, D))


# revision 15
# speedup vs baseline: 1.0652x; 1.0652x over previous
"""Trainium2 Bass kernel for deformable attention.

Contract: kernel(**inputs) takes the FULL inputs (as produced by the problem's
setup_inputs) and returns the FULL [4, 1024, 256] float32 output. Internally the
work is sharded over 8 NeuronCores: core c handles batch c//2 and query half
c%2 (512 queries), with the batch's full value feature map replicated on the
core.

Per-core pipeline (all shapes hardcoded for B=4, Q=1024, D=256, H=W=128,
nh=8, npts=4):
  1. The value projection W_v commutes past the (linear) bilinear/attention
     reduce, so it is folded into the output projection on the host:
     Wcomb_h = W_v @ W_out_h and bvW_h = b_v @ W_out_h, with a per-(q,h)
     sum-of-weights term correcting the bias at zero-padded borders. The
     kernel therefore gathers raw value rows -- no feature-map GEMM.
  2. The value map ships as fp8 e3m4 scaled by 2 (quantization rel err ~1.3%
     end to end, under the 2e-2 gate); the 1/2 is folded into Wcomb. This
     halves gather DMA traffic vs bf16.
  3. Coefficient chain: offsets/attention GEMMs + softmax + bilinear weight
     computation, in [query-partition, sample-free] layout, fp32. Per q-tile
     of 128 queries; gather indices for a tile ship as soon as it finishes.
     All cross-phase intermediates are per-q-tile tiles so chunk 0's gather
     only waits on q-tile 0's index path.
  4. Gather: per (query, head, point, row-corner) descriptor, one dma_gather
     element of 512 fp8 values = two adjacent columns at one row of the
     value map. 2048 idxs per call, 16 calls.
  5. Weighted reduce on the TensorEngine: the 128 gathered slots of a query
     pair are the contraction dim (gathered tile is the fp8 stationary
     operand); the moving operand is a masked block-diagonal [128, 16] bf16
     weight matrix built from bilinear*attention weights. Output lands as
     [d, (q, h)] in PSUM, which is exactly the lhsT layout the final GEMM
     needs. PSUM->SBUF staging runs on the Activation engine to keep DVE off
     the critical path.
  6. out = weighted @ Wcomb + sw * bvW + b_out, stored as bf16; each q-tile's
     output GEMM issues right after its last gather chunk so the PE overlaps
     it with later chunks' DMA.
"""

from contextlib import ExitStack

import numpy as np
import ml_dtypes

NH, NPTS = 8, 4
D = 256
HW = 128            # H == W == 128
NROWS = HW * HW     # 16384
QPC = 512           # queries per core
NCORES = 8
NPAIRS = QPC // 2   # 256 query pairs
NCHUNK = 16         # gather chunks
PAIRS_PER_CHUNK = NPAIRS // NCHUNK       # 16
IDX_PER_CHUNK = PAIRS_PER_CHUNK * 128    # 2048
CHUNKS_PER_QT = NCHUNK // 4              # 4
VSCALE = 2.0        # value map pre-scale folded into Wcomb

_CACHE = {}


def _mask16_np():
    """[128, 16] bf16: mask[qq*64 + h*8 + p*2 + yp, qq*8 + h] = 1."""
    m = np.zeros((128, 16), dtype=np.float32)
    for qq in range(2):
        for h in range(NH):
            for p in range(NPTS):
                for yp in range(2):
                    m[qq * 64 + h * 8 + p * 2 + yp, qq * 8 + h] = 1.0
    return m.astype(ml_dtypes.bfloat16)


def _rep_np():
    """[64, 4, 128] f32: rep[k, g4, 16r+t] = 1 iff k == 16*g4 + t."""
    m = np.zeros((64, 4, 128), dtype=np.float32)
    for g4 in range(4):
        for r in range(8):
            for t in range(16):
                m[16 * g4 + t, g4, 16 * r + t] = 1.0
    return m


def _build_bass():
    import concourse.bass as bass
    import concourse.bacc as bacc
    import concourse.mybir as mybir
    import concourse.tile as tile
    from concourse.masks import make_identity

    f32 = mybir.dt.float32
    bf16 = mybir.dt.bfloat16
    f8 = mybir.dt.float8e3
    i16 = mybir.dt.int16
    i32 = mybir.dt.int32
    Alu = mybir.AluOpType
    Act = mybir.ActivationFunctionType

    nc = bacc.Bacc("TRN2", target_bir_lowering=False,
                   dynamic_dma_scratch_size=65536)

    # ---- I/O ----
    query = nc.dram_tensor("query", [QPC, D], f32, kind="ExternalInput")
    refp = nc.dram_tensor("reference_points", [QPC, 2], f32, kind="ExternalInput")
    value = nc.dram_tensor("value", [NROWS, D], f8, kind="ExternalInput")
    W_off = nc.dram_tensor("W_off", [D, 64], f32, kind="ExternalInput")
    b_off = nc.dram_tensor("b_off", [64], f32, kind="ExternalInput")
    W_attn = nc.dram_tensor("W_attn", [D, 32], f32, kind="ExternalInput")
    b_attn = nc.dram_tensor("b_attn", [32], f32, kind="ExternalInput")
    Wcomb = nc.dram_tensor("Wcomb", [NH * D, D], bf16, kind="ExternalInput")
    bvW = nc.dram_tensor("bvW", [NH, D], bf16, kind="ExternalInput")
    b_out = nc.dram_tensor("b_out", [D], f32, kind="ExternalInput")
    out = nc.dram_tensor("out", [QPC, D], bf16, kind="ExternalOutput")

    mask_dram = nc.inline_tensor(_mask16_np(), name="mask16")
    rep_dram = nc.inline_tensor(_rep_np(), name="rep64")

    with tile.TileContext(nc) as tc, ExitStack() as ctx:
        sb = ctx.enter_context(tc.tile_pool(name="sb", bufs=1))
        ps = ctx.enter_context(tc.tile_pool(name="ps", bufs=1, space="PSUM"))

        # ---- queries for tile 0 first: they head the critical path ----
        q_sbs, rps = [], []
        for qt in range(4):
            q_sbs.append(sb.tile([128, 256], f32, tag=f"q_sb{qt}", name=f"q_sb{qt}"))
            rps.append(sb.tile([128, 2], f32, tag=f"rp{qt}", name=f"rp{qt}"))
        nc.sync.dma_start(q_sbs[0][:], query[0:128, :])
        nc.sync.dma_start(rps[0][:], refp[0:128, :])

        wcat = sb.tile([128, 2, 96], f32, tag="wcat")
        nc.sync.dma_start(wcat[:, :, 0:64], W_off[:].rearrange("(t p) n -> p t n", p=128))
        nc.sync.dma_start(wcat[:, :, 64:96], W_attn[:].rearrange("(t p) n -> p t n", p=128))
        nc.vector.tensor_scalar_mul(wcat[:, :, 0:64], wcat[:, :, 0:64], 0.1)
        bias_cat = sb.tile([1, 96], f32, tag="bias_cat")
        nc.sync.dma_start(bias_cat[:, 0:64], b_off[None, :])
        nc.sync.dma_start(bias_cat[:, 64:96], b_attn[None, :])
        nc.vector.tensor_scalar_mul(bias_cat[:, 0:64], bias_cat[:, 0:64], 0.1)

        rep64 = sb.tile([64, 4, 128], f32, tag="rep64")
        nc.sync.dma_start(rep64[:], rep_dram[:])

        # wout's 2.9us transfer must land inside the lead-in DMA idle, so it
        # goes on the SP queue right behind the critical-path loads.
        wout_bf = sb.tile([128, 16, 256], bf16, tag="wout")
        nc.sync.dma_start(wout_bf[:], Wcomb[:].rearrange("(t p) n -> p t n", p=128))

        for qt in range(1, 4):
            qsl = slice(qt * 128, (qt + 1) * 128)
            nc.sync.dma_start(q_sbs[qt][:], query[qsl, :])
            nc.sync.dma_start(rps[qt][:], refp[qsl, :])

        ident = sb.tile([128, 128], f32, tag="ident")
        make_identity(nc, ident[:])
        ones1 = sb.tile([1, 128], f32, tag="ones1")
        nc.vector.memset(ones1[:], 1.0)
        # warm the activation table off the critical path
        actwarm = sb.tile([1, 128], f32, tag="actwarm")
        nc.scalar.activation(actwarm[:], ones1[:], Act.Exp)
        mask16 = sb.tile([128, 16], bf16, tag="mask16")
        nc.sync.dma_start(mask16[:], mask_dram[:])
        bvw_bf = sb.tile([8, 256], bf16, tag="bvw")
        nc.sync.dma_start(bvw_bf[:], bvW[:])
        bout_sb = sb.tile([1, 256], f32, tag="bout")
        nc.sync.dma_start(bout_sb[:], b_out[None, :])

        # per-q-tile persistent intermediates (split so consumers only wait
        # on their own tile's producers)
        idxts = [sb.tile([128, 64, 2, 4], i16, tag=f"idxt{qt}", name=f"idxt{qt}")
                 for qt in range(4)]
        w_a_is = [sb.tile([128, 64], bf16, tag=f"w_a{qt}", name=f"w_a{qt}")
                  for qt in range(4)]
        w_b_is = [sb.tile([128, 64], bf16, tag=f"w_b{qt}", name=f"w_b{qt}")
                  for qt in range(4)]
        reds = [sb.tile([128, 2, 128, 8], bf16, tag=f"red{qt}", name=f"red{qt}")
                for qt in range(4)]
        swTs = [sb.tile([8, 128], bf16, tag=f"swT{qt}", name=f"swT{qt}")
                for qt in range(4)]

        # ================= coefficient phase (4 q-tiles of 128) =============
        # per q-tile: offsets GEMM + gather-index path (ships indices as
        # early as possible), weight path as a separate pass; interleaved so
        # late tiles' indices land before the gather stream catches up while
        # tile 0's weights are still ready before its first reduce.
        qt_state = {}

        def idx_pass(qt):
            q_sb, rp = q_sbs[qt], rps[qt]

            pst = ps.tile([128, 256], f32, tag="tp", bufs=2)
            nc.tensor.transpose(pst[:, 0:128], q_sb[:, 0:128], ident[:])
            nc.tensor.transpose(pst[:, 128:256], q_sb[:, 128:256], ident[:])
            qT = sb.tile([128, 2, 128], f32, tag="qT", bufs=2)
            nc.scalar.copy(qT[:], pst[:])

            psc = ps.tile([128, 96], f32, tag="tp", bufs=2)
            nc.tensor.matmul(psc[:], qT[:, 0, :], wcat[:, 0, :], start=True, stop=False)
            nc.tensor.matmul(psc[:], qT[:, 1, :], wcat[:, 1, :], start=False, stop=False)
            nc.tensor.matmul(psc[:], ones1[:], bias_cat[:], start=False, stop=True)
            coef = sb.tile([128, 96], f32, tag="coef_sb", bufs=4)
            nc.scalar.copy(coef[:], psc[:])

            # sampling grid -> bilinear weights, x/y interleaved [128, 32, 2]
            t_u = sb.tile([128, 32, 2], f32, tag="t_u", bufs=4)
            nc.vector.tensor_tensor(
                t_u[:], coef[:, 0:64].rearrange("p (s c) -> p s c", c=2),
                rp[:, None, :].to_broadcast([128, 32, 2]), Alu.add)
            t_c = t_u  # in-place ok per-element
            nc.vector.tensor_scalar(t_c[:], t_u[:], 0.0, 1.0, Alu.max, Alu.min)
            pxs = sb.tile([128, 64], f32, tag="pxs", bufs=4)  # px + 128
            nc.vector.tensor_scalar(pxs[:], t_c[:].rearrange("p a b -> p (a b)"),
                                    128.0, 127.5, Alu.mult, Alu.add)
            # floor(pxs) = round(pxs - 0.5): pxs is positive and lands on
            # k+0.5 only at the clip boundaries 127.5/255.5, where round
            # gives 127/255 = floor as required.
            ph = sb.tile([128, 64], f32, tag="ph", bufs=4)
            nc.vector.tensor_scalar_add(ph[:], pxs[:], -0.5)
            ri = sb.tile([128, 64], i32, tag="ri", bufs=4)
            nc.vector.tensor_copy(ri[:], ph[:])
            flr = sb.tile([128, 64], f32, tag="flr", bufs=4)  # floor(px) + 128
            nc.vector.tensor_copy(flr[:], ri[:])
            st = sb.tile([128, 64], f32, tag="st", bufs=4)    # clip start + 128
            nc.vector.tensor_scalar(st[:], flr[:], 128.0, 254.0, Alu.max, Alu.min)
            # ---- gather-index path first: this q-tile's gathers can start
            # while the weight path below is still computing ----
            tbase = sb.tile([128, 32], f32, tag="tbase", bufs=4)
            nc.vector.tensor_scalar(
                tbase[:], st[:].rearrange("p (s c) -> p s c", c=2)[:, :, 1],
                128.0, -16512.0, Alu.mult, Alu.add)
            idx64 = sb.tile([128, 32, 2], f32, tag="idx64", bufs=4)
            nc.vector.tensor_tensor(idx64[:, :, 0], tbase[:],
                                    st[:].rearrange("p (s c) -> p s c", c=2)[:, :, 0], Alu.add)
            nc.vector.tensor_scalar_add(idx64[:, :, 1], idx64[:, :, 0], 128.0)
            # transpose idx to [slot, q], then replicate to all 8
            # 16-partition blocks (one per Q7 core) with 0/1 REP matmuls:
            # out[16r+t, q] = ivT[16*g4+t, q]
            iv = idx64[:].rearrange("p s c -> p (s c)")
            psv = ps.tile([64, 128], f32, tag="psv", bufs=1)
            nc.tensor.transpose(psv[:], iv[:], ident[:])
            ivT = sb.tile([64, 128], f32, tag="ivT", bufs=2)
            nc.vector.tensor_copy(ivT[:], psv[:])
            psi = ps.tile([128, 4, 128], f32, tag="psi", bufs=1)
            for g4 in range(4):
                nc.tensor.matmul(psi[:, g4, :], rep64[:, g4, :], ivT[:],
                                 start=True, stop=True)
            # one DVE copy: [128, (g4, j, qq)] f32 -> idxt[., j, qq, g4] i16
            nc.vector.tensor_copy(
                idxts[qt][:],
                psi[:].rearrange("p g (j q) -> p j q g", q=2))
            qt_state[qt] = (coef, pxs, flr, st)

        def weight_pass(qt):
            coef, pxs, flr, st = qt_state[qt]
            # softmax + bilinear weights
            expw = sb.tile([128, 8, 4], f32, tag="expw", bufs=4)
            nc.scalar.activation(expw[:], coef[:, 64:96], Act.Exp)
            den = sb.tile([128, 8], f32, tag="den", bufs=4)
            nc.vector.tensor_reduce(den[:], expw[:], axis=mybir.AxisListType.X, op=Alu.add)
            rden = sb.tile([128, 8], f32, tag="rden", bufs=4)
            nc.vector.reciprocal(rden[:], den[:])
            attn = sb.tile([128, 32], f32, tag="attn", bufs=4)
            nc.vector.tensor_tensor(
                attn[:].rearrange("p (h f) -> p h f", f=4), expw[:],
                rden[:, :, None].to_broadcast([128, 8, 4]), Alu.mult)
            w1 = sb.tile([128, 64], f32, tag="w1", bufs=4)
            nc.vector.tensor_tensor(w1[:], pxs[:], flr[:], Alu.subtract)
            dd = sb.tile([128, 64], f32, tag="dd", bufs=4)
            nc.vector.tensor_tensor(dd[:], flr[:], st[:], Alu.subtract)
            m0 = sb.tile([128, 64], f32, tag="m0", bufs=4)
            nc.vector.tensor_scalar(m0[:], dd[:], 0.0, None, Alu.is_equal)
            mneg = sb.tile([128, 64], f32, tag="mneg", bufs=4)
            nc.vector.tensor_scalar(mneg[:], dd[:], -1.0, None, Alu.is_equal)
            mpos = sb.tile([128, 64], f32, tag="mpos", bufs=4)
            nc.vector.tensor_scalar(mpos[:], dd[:], 1.0, None, Alu.is_equal)
            u0 = sb.tile([128, 64], f32, tag="u0", bufs=4)
            nc.vector.tensor_scalar(u0[:], w1[:], 1.0, -1.0, Alu.subtract, Alu.mult)
            # wA = u0*m0 + u1*mneg ; wB = u1*m0 + u0*mpos   (u1 == w1)
            tA = sb.tile([128, 64], f32, tag="tA", bufs=4)
            nc.vector.tensor_tensor(tA[:], u0[:], m0[:], Alu.mult)
            tB = sb.tile([128, 64], f32, tag="tB", bufs=4)
            nc.vector.tensor_tensor(tB[:], w1[:], mneg[:], Alu.mult)
            wA = sb.tile([128, 32, 2], f32, tag="wA", bufs=4)
            nc.vector.tensor_tensor(wA[:].rearrange("p a b -> p (a b)"), tA[:], tB[:], Alu.add)
            nc.vector.tensor_tensor(tA[:], w1[:], m0[:], Alu.mult)
            nc.vector.tensor_tensor(tB[:], u0[:], mpos[:], Alu.mult)
            wB = sb.tile([128, 32, 2], f32, tag="wB", bufs=4)
            nc.vector.tensor_tensor(wB[:].rearrange("p a b -> p (a b)"), tA[:], tB[:], Alu.add)

            # combine with attention; build wab [128, (AB, h*p, yp)]
            aw = sb.tile([128, 32], f32, tag="aw", bufs=4)
            nc.vector.tensor_tensor(aw[:], attn[:], wA[:, :, 0], Alu.mult)
            bw = sb.tile([128, 32], f32, tag="bw", bufs=4)
            nc.vector.tensor_tensor(bw[:], attn[:], wB[:, :, 0], Alu.mult)
            vcat = sb.tile([128, 32, 2], f32, tag="vcat", bufs=4)
            nc.vector.tensor_copy(vcat[:, :, 0], wA[:, :, 1])
            nc.vector.tensor_copy(vcat[:, :, 1], wB[:, :, 1])
            wab = sb.tile([128, 2, 32, 2], f32, tag="wab", bufs=4)
            nc.vector.tensor_tensor(wab[:, 0], vcat[:],
                                    aw[:, :, None].to_broadcast([128, 32, 2]), Alu.mult)
            nc.vector.tensor_tensor(wab[:, 1], vcat[:],
                                    bw[:, :, None].to_broadcast([128, 32, 2]), Alu.mult)

            # sum of all weights per (q, h) -- border-clip correction for the
            # folded b_v term: sw = sum_{AB,p,yp} wab
            swq = sb.tile([128, 8], f32, tag="swq", bufs=4)
            nc.vector.tensor_reduce(
                swq[:], wab[:].rearrange("p a (h r) c -> p h a r c", h=8),
                axis=mybir.AxisListType.XYZ, op=Alu.add)

            pst2b = ps.tile([128, 256], f32, tag="tp", bufs=2)
            nc.tensor.transpose(pst2b[:, 0:128],
                                wab[:].rearrange("p a s c -> p (a s c)"), ident[:])
            pst3 = ps.tile([8, 128], f32, tag="img", bufs=2)
            nc.tensor.transpose(pst3[:], swq[:], ident[:])
            wabT = sb.tile([128, 128], f32, tag="wabT", bufs=2)
            nc.scalar.copy(wabT[:], pst2b[:, 0:128])
            nc.scalar.copy(swTs[qt][:], pst3[:])
            nc.vector.tensor_copy(w_a_is[qt][0:64, :], wabT[0:64, 0:128:2])
            nc.vector.tensor_copy(w_a_is[qt][64:128, :], wabT[0:64, 1:128:2])
            nc.vector.tensor_copy(w_b_is[qt][0:64, :], wabT[64:128, 0:128:2])
            nc.vector.tensor_copy(w_b_is[qt][64:128, :], wabT[64:128, 1:128:2])

        idx_pass(0)
        idx_pass(1)
        weight_pass(0)
        idx_pass(2)
        weight_pass(1)
        idx_pass(3)
        weight_pass(2)
        weight_pass(3)

        # ================= gather + weighted reduce =========================
        import concourse.bass as bass_mod
        gather_src = bass_mod.AP(
            tensor=value, offset=0, ap=[[256, NROWS - 1], [1, 512]])

        def final_gemm(qt):
            pso = ps.tile([128, 256], f32, tag="img", bufs=2)
            for kt in range(16):
                h, dh = kt // 2, kt % 2
                lhsT = reds[qt][:, dh, :, h]
                nc.tensor.matmul(pso[:], lhsT, wout_bf[:, kt, :],
                                 start=(kt == 0), stop=False)
            nc.tensor.matmul(pso[:], swTs[qt][:], bvw_bf[:],
                             start=False, stop=False)
            nc.tensor.matmul(pso[:], ones1[:], bout_sb[:], start=False, stop=True)
            o_sb = sb.tile([128, 256], bf16, tag="o_sb", bufs=2)
            nc.scalar.copy(o_sb[:], pso[:])
            nc.sync.dma_start(out[qt * 128:(qt + 1) * 128, :], o_sb[:])

        for g in range(NCHUNK):
            qt, gq = g // CHUNKS_PER_QT, g % CHUNKS_PER_QT
            gsl = slice(gq * PAIRS_PER_CHUNK, (gq + 1) * PAIRS_PER_CHUNK)
            gt_sb = sb.tile([128, PAIRS_PER_CHUNK, 512], f8, tag="gat", bufs=5)
            nc.gpsimd.dma_gather(
                out_ap=gt_sb[:],
                in_ap=gather_src,
                idxs_ap=idxts[qt][:].rearrange("p a b c -> p (a b c)")[
                    :, gq * (IDX_PER_CHUNK // 16):(gq + 1) * (IDX_PER_CHUNK // 16)],
                num_idxs=IDX_PER_CHUNK,
                num_idxs_reg=IDX_PER_CHUNK,
                elem_size=512,
                elem_step=256,
            )
            wblkA = sb.tile([128, PAIRS_PER_CHUNK, 16], bf16, tag="wblkA", bufs=4)
            nc.vector.tensor_tensor(
                wblkA[:], mask16[:, None, :].to_broadcast([128, PAIRS_PER_CHUNK, 16]),
                w_a_is[qt][:, gsl, None].to_broadcast([128, PAIRS_PER_CHUNK, 16]),
                Alu.mult)
            wblkB = sb.tile([128, PAIRS_PER_CHUNK, 16], bf16, tag="wblkB", bufs=4)
            nc.vector.tensor_tensor(
                wblkB[:], mask16[:, None, :].to_broadcast([128, PAIRS_PER_CHUNK, 16]),
                w_b_is[qt][:, gsl, None].to_broadcast([128, PAIRS_PER_CHUNK, 16]),
                Alu.mult)

            pred = ps.tile([128, 2, PAIRS_PER_CHUNK * 16], f32, tag="red_ps", bufs=2)
            for j in range(PAIRS_PER_CHUNK):
                osl = slice(j * 16, (j + 1) * 16)
                nc.tensor.matmul(pred[:, 0, osl], gt_sb[:, j, 0:128], wblkA[:, j, :],
                                 start=True, stop=False)
                nc.tensor.matmul(pred[:, 1, osl], gt_sb[:, j, 128:256], wblkA[:, j, :],
                                 start=True, stop=False)
                nc.tensor.matmul(pred[:, 0, osl], gt_sb[:, j, 256:384], wblkB[:, j, :],
                                 start=False, stop=True)
                nc.tensor.matmul(pred[:, 1, osl], gt_sb[:, j, 384:512], wblkB[:, j, :],
                                 start=False, stop=True)
            qsl2 = slice(gq * 2 * PAIRS_PER_CHUNK, (gq + 1) * 2 * PAIRS_PER_CHUNK)
            nc.scalar.copy(
                reds[qt][:, :, qsl2, :],
                pred[:].rearrange("p d (a b) -> p d a b", b=8))
            if gq == CHUNKS_PER_QT - 1:
                final_gemm(qt)

    nc.compile()
    return nc


def _get_nc():
    if "nc" not in _CACHE:
        _CACHE["nc"] = _build_bass()
    return _CACHE["nc"]


def _make_in_maps(inputs):
    query = np.ascontiguousarray(np.asarray(inputs["query"], dtype=np.float32))
    refp = np.ascontiguousarray(np.asarray(inputs["reference_points"], dtype=np.float32))
    value = np.ascontiguousarray(
        (np.asarray(inputs["value"], dtype=np.float32) * VSCALE
         ).astype(ml_dtypes.float8_e3m4))
    consts = {
        k: np.ascontiguousarray(np.asarray(inputs[k], np.float32))
        for k in ["W_off", "b_off", "W_attn", "b_attn", "b_out"]
    }
    W_v = np.asarray(inputs["W_v"], np.float64)
    b_v = np.asarray(inputs["b_v"], np.float64)
    W_out = np.asarray(inputs["W_out"], np.float64).reshape(NH, D, D)
    consts["Wcomb"] = np.ascontiguousarray(
        (np.einsum("ij,hjk->hik", W_v, W_out) / VSCALE
         ).reshape(NH * D, D).astype(ml_dtypes.bfloat16))
    consts["bvW"] = np.ascontiguousarray(
        np.einsum("j,hjk->hk", b_v, W_out).astype(ml_dtypes.bfloat16))
    in_maps = []
    for c in range(NCORES):
        b, s = c // 2, c % 2
        qsl = slice(s * QPC, (s + 1) * QPC)
        in_maps.append({
            "query": np.ascontiguousarray(query[b, qsl]),
            "reference_points": np.ascontiguousarray(refp[b, qsl]),
            "value": np.ascontiguousarray(value[b]),
            **consts,
        })
    return in_maps


def _assemble(outs, shape):
    out = np.zeros(shape, dtype=np.float32)
    for c in range(NCORES):
        b, s = c // 2, c % 2
        out[b, s * QPC:(s + 1) * QPC] = np.asarray(outs[c]["out"], dtype=np.float32)
    return out


def kernel(query, reference_points, value, W_off, b_off, W_attn, b_attn,
           W_v, b_v, W_out, b_out, H=128, W=128, **_unused):
    assert int(H) == HW and int(W) == HW
    from concourse.bass_utils import run_bass_kernel_spmd

    inputs = dict(query=query, reference_points=reference_points, value=value,
                  W_off=W_off, b_off=b_off, W_attn=W_attn, b_attn=b_attn,
                  W_v=W_v, b_v=b_v, W_out=W_out, b_out=b_out)
    in_maps = _make_in_maps(inputs)
    nc = _get_nc()
    res = run_bass_kernel_spmd(nc, in_maps, core_ids=list(range(NCORES)))
    outs = res.results if hasattr(res, "results") else res
    B, Q, _ = np.asarray(query).shape
    return _assemble(outs, (B, Q, D))
